# revision 1
# baseline (speedup 1.0000x reference)
"""Trainium2 Bass kernel for nn_HSL_Layer_Part1 (GNN message passing).

Computes, for X:(512,128) V,E:(8192,) int64, MLP weights W1:(256,256) b1 W2 b2:
    eX   = segment_mean(X[V], E, 512)                      # (512,128)
    hX   = X @ W1[:, :128].T                               # (512,256)
    hE   = eX @ W1[:, 128:].T                              # (512,256)
    prob = clip(sigmoid(relu(hX[:,None,:] + hE[None,:,:] + b1) @ W2[0] + b2))

Distribution: 8 cores, sharded over the 512 edges (64 edges/core).  Each core
computes the full (512 nodes x 64 edges) output block in transposed (m, n)
layout; the host reassembles prob[n, m].

The segment-mean is reformulated as a dense matmul: the host builds (from the
integer index tensors V/E only) the normalized incidence-count matrix
A_norm[m, n] = count(E==m & V==n) / max(count(E==m), 1), so eX = A_norm @ X is
computed on-device by the tensor engine.

Per-core device program:
  setup:  eX_T = X.T @ A_norm_c.T          (PE, fp32)
          hE_T = W1b @ eX_T; B = hE_T + b1 (PE + DVE, fp32)     (256h x 64m)
          hX_T = W1a @ X.T -> bf16         (PE, fp32 -> bf16)   (256h x 512n)
  main:   for each local edge m, h-block hb:
             T = relu(hX_T[hb] + B[hb][:, m])   (DVE tensor_scalar bf16 4x,
                                                 some tiles on ACT)
             psum[m,:] += W2[hb].T @ T          (PE, M=1, col-tiled 4-wide)
          per 4-edge group: sigmoid(psum + b2) on ACT (junk rows included),
          DMA the 4 valid rows into the packed (64, 512) output tile.
  tail:   clip to [1e-6, 1-1e-6] (DVE), store.
"""

import numpy as np

NUM_NODES = 512
NUM_EDGES = 512
EMB = 128
HID = 256
N_CORES = 8
M_LOC = NUM_EDGES // N_CORES  # 64 edges per core

# every ACT_EVERY-th relu tile runs on the scalar engine instead of DVE
ACT_EVERY = 9

_CACHE = {}
LAST_RESULTS = None  # bass results object of the most recent run (for profiling)


def _build_program():
    import concourse.bacc as bacc
    import concourse.mybir as mybir
    import concourse.tile as tile

    f32 = mybir.dt.float32
    bf16 = mybir.dt.bfloat16
    Relu = mybir.ActivationFunctionType.Relu
    Sigmoid = mybir.ActivationFunctionType.Sigmoid
    Alu = mybir.AluOpType

    nc = bacc.Bacc(
        "TRN2", target_bir_lowering=False, debug=False, num_devices=N_CORES
    )

    X_e = nc.dram_tensor("X", [NUM_NODES, EMB], f32, kind="ExternalInput").ap()
    XT_e = nc.dram_tensor("XT", [EMB, NUM_NODES], f32, kind="ExternalInput").ap()
    W1aT_e = nc.dram_tensor("W1aT", [EMB, HID], f32, kind="ExternalInput").ap()
    W1bT_e = nc.dram_tensor("W1bT", [EMB, HID], f32, kind="ExternalInput").ap()
    W2b_e = nc.dram_tensor("W2b", [EMB, 2], bf16, kind="ExternalInput").ap()
    b1c_e = nc.dram_tensor("b1c", [EMB, 2], f32, kind="ExternalInput").ap()
    b2c_e = nc.dram_tensor("b2c", [EMB, 1], f32, kind="ExternalInput").ap()
    AT_e = nc.dram_tensor("AT", [NUM_NODES, M_LOC], f32, kind="ExternalInput").ap()
    out_e = nc.dram_tensor(
        "out", [M_LOC, NUM_NODES], f32, kind="ExternalOutput"
    ).ap()

    KB = NUM_NODES // 128  # 4 K-blocks over nodes

    with tile.TileContext(nc) as tc:
        with (
            tc.tile_pool(name="const", bufs=1) as cpool,
            tc.tile_pool(name="tpool", bufs=6) as tpool,
            tc.tile_pool(name="gpool", bufs=3) as gpool,
            tc.tile_pool(name="pset", bufs=3, space="PSUM") as pset,
            tc.tile_pool(name="pgrp", bufs=4, space="PSUM") as pgrp,
        ):
            # ---- input loads -------------------------------------------------
            X_sb = cpool.tile([128, KB, EMB], f32, tag="X")
            nc.sync.dma_start(out=X_sb[:], in_=X_e.rearrange("(o p) d -> p o d", p=128))
            AT_sb = cpool.tile([128, KB, M_LOC], f32, tag="AT")
            nc.sync.dma_start(
                out=AT_sb[:], in_=AT_e.rearrange("(o p) m -> p o m", p=128)
            )
            XT_sb = cpool.tile([EMB, NUM_NODES], f32, tag="XT")
            nc.sync.dma_start(out=XT_sb[:], in_=XT_e[:])
            W1aT_sb = cpool.tile([EMB, HID], f32, tag="W1aT")
            nc.sync.dma_start(out=W1aT_sb[:], in_=W1aT_e[:])
            W1bT_sb = cpool.tile([EMB, HID], f32, tag="W1bT")
            nc.sync.dma_start(out=W1bT_sb[:], in_=W1bT_e[:])
            W2b_sb = cpool.tile([EMB, 2], bf16, tag="W2b")
            nc.sync.dma_start(out=W2b_sb[:], in_=W2b_e[:])
            b1c_sb = cpool.tile([EMB, 2], f32, tag="b1c")
            nc.sync.dma_start(out=b1c_sb[:], in_=b1c_e[:])
            b2c_sb = cpool.tile([EMB, 1], f32, tag="b2c")
            nc.sync.dma_start(out=b2c_sb[:], in_=b2c_e[:])

            # ---- eX_T = X.T @ A_norm_c.T  (128d x 64m, fp32) -----------------
            ps_eX = pset.tile([128, 512], f32, tag="ps")
            for kb in range(KB):
                nc.tensor.matmul(
                    out=ps_eX[:, :M_LOC],
                    lhsT=X_sb[:, kb, :],
                    rhs=AT_sb[:, kb, :],
                    start=(kb == 0),
                    stop=(kb == KB - 1),
                )
            eX_sb = cpool.tile([128, M_LOC], f32, tag="eX")
            nc.vector.tensor_copy(out=eX_sb[:], in_=ps_eX[:, :M_LOC])

            # ---- B[hb] = W1b @ eX_T + b1  (2 x (128h x 64m), fp32) -----------
            B_sb = []
            for hb in range(2):
                ps_hE = pset.tile([128, 512], f32, tag="ps")
                nc.tensor.matmul(
                    out=ps_hE[:, :M_LOC],
                    lhsT=W1bT_sb[:, hb * 128 : (hb + 1) * 128],
                    rhs=eX_sb[:],
                    start=True,
                    stop=True,
                )
                Bt = cpool.tile([128, M_LOC], f32, tag=f"B{hb}")
                nc.vector.tensor_scalar(
                    out=Bt[:],
                    in0=ps_hE[:, :M_LOC],
                    scalar1=b1c_sb[:, hb : hb + 1],
                    scalar2=None,
                    op0=Alu.add,
                )
                B_sb.append(Bt)

            # ---- hX_T[hb] = W1a @ X.T  (2 x (128h x 512n), bf16) -------------
            hXT_sb = []
            for hb in range(2):
                ps_hX = pset.tile([128, 512], f32, tag="ps")
                nc.tensor.matmul(
                    out=ps_hX[:],
                    lhsT=W1aT_sb[:, hb * 128 : (hb + 1) * 128],
                    rhs=XT_sb[:],
                    start=True,
                    stop=True,
                )
                hXt = cpool.tile([128, NUM_NODES], bf16, tag=f"hXT{hb}")
                nc.vector.tensor_copy(out=hXt[:], in_=ps_hX[:])
                hXT_sb.append(hXt)

            # ---- main loop: 16 groups x 4 edges x 2 h-blocks -----------------
            probs_sb = cpool.tile([M_LOC, NUM_NODES], f32, tag="probs")
            ui = 0
            for g in range(M_LOC // 4):
                ps_grp = pgrp.tile([128, 512], f32, tag="grp")
                for j in range(4):
                    m = 4 * g + j
                    for hb in range(2):
                        T = tpool.tile([128, NUM_NODES], bf16, tag="T")
                        if ui % ACT_EVERY == ACT_EVERY - 1:
                            nc.scalar.activation(
                                out=T[:],
                                in_=hXT_sb[hb][:],
                                func=Relu,
                                bias=B_sb[hb][:, m : m + 1],
                            )
                        else:
                            nc.vector.tensor_scalar(
                                out=T[:],
                                in0=hXT_sb[hb][:],
                                scalar1=B_sb[hb][:, m : m + 1],
                                scalar2=0.0,
                                op0=Alu.add,
                                op1=Alu.max,
                            )
                        ui += 1
                        nc.tensor.matmul(
                            out=ps_grp[32 * j : 32 * j + 1, :],
                            lhsT=W2b_sb[:, hb : hb + 1],
                            rhs=T[:],
                            start=(hb == 0),
                            stop=(hb == 1),
                            tile_position=(0, 32 * j),
                        )
                # sigmoid(logits + b2) for the whole bank (junk rows too)
                prob_grp = gpool.tile([128, NUM_NODES], f32, tag="pg")
                nc.scalar.activation(
                    out=prob_grp[:],
                    in_=ps_grp[:],
                    func=Sigmoid,
                    bias=b2c_sb[:, 0:1],
                )
                for j in range(4):
                    nc.sync.dma_start(
                        out=probs_sb[4 * g + j : 4 * g + j + 1, :],
                        in_=prob_grp[32 * j : 32 * j + 1, :],
                    )

            # ---- clip + store ------------------------------------------------
            out_sb = cpool.tile([M_LOC, NUM_NODES], f32, tag="out")
            nc.vector.tensor_scalar(
                out=out_sb[:],
                in0=probs_sb[:],
                scalar1=1.0 - 1e-6,
                scalar2=1e-6,
                op0=Alu.min,
                op1=Alu.max,
            )
            nc.sync.dma_start(out=out_e[:], in_=out_sb[:])

    nc.finalize()
    return nc


def kernel(X, V, E, W1, b1, W2, b2):
    import ml_dtypes
    from concourse.bass_utils import run_bass_kernel_spmd

    global LAST_RESULTS

    X = np.asarray(X, dtype=np.float32)
    V = np.asarray(V).astype(np.int64)
    E = np.asarray(E).astype(np.int64)
    W1 = np.asarray(W1, dtype=np.float32)
    b1 = np.asarray(b1, dtype=np.float32)
    W2 = np.asarray(W2, dtype=np.float32)
    b2 = np.asarray(b2, dtype=np.float32)

    # host-side index preprocessing: incidence-count matrix, row-normalized
    A = np.zeros((NUM_EDGES, NUM_NODES), dtype=np.float32)
    np.add.at(A, (E, V), 1.0)
    cnt = A.sum(axis=1)
    A_norm = A / np.maximum(cnt, 1.0)[:, None]

    XT = np.ascontiguousarray(X.T)
    W1aT = np.ascontiguousarray(W1[:, :EMB].T)
    W1bT = np.ascontiguousarray(W1[:, EMB:].T)
    W2b = np.ascontiguousarray(W2[0].reshape(2, EMB).T).astype(ml_dtypes.bfloat16)
    b1c = np.ascontiguousarray(b1.reshape(2, EMB).T)
    b2c = np.full((EMB, 1), float(b2[0]), dtype=np.float32)

    if "nc" not in _CACHE:
        _CACHE["nc"] = _build_program()
    nc = _CACHE["nc"]

    in_maps = []
    for c in range(N_CORES):
        AT_c = np.ascontiguousarray(
            A_norm[c * M_LOC : (c + 1) * M_LOC, :].T
        )  # (512, 64)
        in_maps.append(
            {
                "X": X,
                "XT": XT,
                "W1aT": W1aT,
                "W1bT": W1bT,
                "W2b": W2b,
                "b1c": b1c,
                "b2c": b2c,
                "AT": AT_c,
            }
        )

    res = run_bass_kernel_spmd(nc, in_maps, list(range(N_CORES)))
    LAST_RESULTS = res

    out = np.empty((NUM_NODES, NUM_EDGES), dtype=np.float32)
    for c in range(N_CORES):
        out[:, c * M_LOC : (c + 1) * M_LOC] = res.results[c]["out"].T
    return out

